# revision 13
# baseline (speedup 1.0000x reference)
"""TRN2 Bass kernel for nn_CombinedLoraA (moe_routing).

Computation: out[c, 0, r] = sum_k x[xids[c*64+r], 0, k] * A[wids[c], k, r]
  x: (512, 1, 4096) f32, xids: (20480,) i32, wids: (320,) i32, A: (80, 4096, 64) f32
  out: (320, 1, 64) f32

Strategy (adapter-parallel across 8 cores, routing baked in at trace time):
  - Host assigns exactly 10 adapters to each core (greedy row-count balance).
  - Each core computes the dense precompute P[w, t, r] = sum_k X[t, k] A[w, k, r]
    for ALL 512 tokens x its 10 adapters on the PE (X^T stationary, adapters'
    columns concatenated in the moving operand). X^T is transposed on the host.
  - P is copied PSUM->SBUF->DRAM per 128-token chunk (4 separate DRAM tables
    so extraction for chunk t overlaps chunk t+1's matmuls); the needed
    out[c, r] = P[w_c, tok[c, r], r] elements are fetched with one dma_gather
    per chunk (64-f32 rows, int16 indices), then a host-baked one-hot mask
    multiply + free-axis reduce picks the right column per (c, r) pair.
  - Host stitches the 8 per-core staging buffers into the (320, 1, 64) output.

Precision modes for the big matmul (error measured on HW at K=4096):
  float32 (3e-7), split bf16 hi/lo 3-matmul (4.5e-6), float32r (1.7e-4),
  bf16 (2.4e-3).
"""

import os
import sys

sys.path.insert(0, "/opt/trn_rl_repo")

import numpy as np
import ml_dtypes

import concourse.tile as tile
from concourse import mybir, bacc
from concourse.bass_utils import run_bass_kernel_spmd

BATCH, C, R, K, NA = 512, 320, 64, 4096, 80
NCORES = 8
NW = NA // NCORES  # 10 adapters per core
KC = K // 128  # 32 contraction chunks
TC = BATCH // 128  # 4 token chunks
NFREE = NW * R  # 640 moving columns in the main matmul
PIECES = [1, 1] + [2] * 15  # kc-sized DMA pieces (sum = KC)

SPLIT_KC = int(os.environ.get("SPLIT_KC", "20"))
MODE = os.environ.get("KERNEL_MODE", "bf16")  # float32 | split | float32r | bf16
REPS = int(os.environ.get("BENCH_REPS", "1"))
MASK_BF16 = os.environ.get("MASK_BF16", "1") == "1"


def _plan(wids: np.ndarray):
    """Assign exactly NW (adapter, row-sublist) slots per core, balancing row
    count. An adapter with more than ROWCAP rows is split across slots (its
    weights are simply duplicated in the per-core A buffer), which bounds any
    core's row count and therefore the extraction buffer sizes even for
    degenerate wids distributions. For uniform wids (max rows/adapter well
    under ROWCAP) this reduces to one slot per adapter."""
    ROWCAP = 40
    rows_of = [[] for _ in range(NA)]
    for c, w in enumerate(wids):
        rows_of[w].append(c)
    slots = []  # (w, row sublist)
    for w in range(NA):
        rows = rows_of[w]
        if not rows:
            slots.append((w, []))
        else:
            for i in range(0, len(rows), ROWCAP):
                slots.append((w, rows[i : i + ROWCAP]))
    # keep exactly NCORES*NW slots: drop empty ones if over, pad if under
    empty = [s for s in slots if not s[1]]
    full = [s for s in slots if s[1]]
    assert len(full) <= NCORES * NW, "too many hot adapters to split"
    slots = full + empty[: NCORES * NW - len(full)]
    while len(slots) < NCORES * NW:
        slots.append((0, []))
    order = sorted(range(len(slots)), key=lambda s: -len(slots[s][1]))
    core_slots = [[] for _ in range(NCORES)]
    core_load = [0] * NCORES
    for s in order:
        cands = [i for i in range(NCORES) if len(core_slots[i]) < NW]
        i = min(cands, key=lambda j: core_load[j])
        core_slots[i].append(slots[s])
        core_load[i] += len(slots[s][1])
    return core_slots


def _chunks():
    n = int(os.environ.get("NCHUNKS", "2"))
    w = NFREE // n
    return tuple((i * w, (i + 1) * w) for i in range(n))


def _mode_cfg():
    f32, bf16 = mybir.dt.float32, mybir.dt.bfloat16
    if MODE == "float32":
        return f32, 1, _chunks()
    if MODE == "float32r":
        return mybir.dt.float32r, 1, _chunks()
    if MODE == "bf16":
        return bf16, 1, _chunks()
    return bf16, 2, _chunks()  # split


def _build_bass(gchunks: list[int]):
    """gchunks[t] = number of 128-row gather chunks for token-chunk group t."""
    nc = bacc.Bacc("TRN2", target_bir_lowering=False, num_swdge_queues=2)
    f32 = mybir.dt.float32
    mdt = mybir.dt.bfloat16 if MASK_BF16 else f32
    din, halves, chunks = _mode_cfg()
    nchunk = sum(gchunks)

    xt_d = [
        nc.dram_tensor(f"xt{h}", [K, BATCH], din, kind="ExternalInput")
        for h in range(halves)
    ]
    ac_d = [
        nc.dram_tensor(f"ac{h}", [KC, 128, NFREE], din, kind="ExternalInput")
        for h in range(halves)
    ]
    gcols = [gc * 8 for gc in gchunks]  # idx columns per group (num_idxs/16)
    gidx_d = nc.dram_tensor(
        "gidx", [128, sum(gcols)], mybir.dt.int16, kind="ExternalInput"
    )
    mask_d = nc.dram_tensor("mask", [128, nchunk, R], mdt, kind="ExternalInput")
    out_d = nc.dram_tensor("out", [128, nchunk], f32, kind="ExternalOutput")

    with tile.TileContext(nc) as tc:
        with (
            tc.tile_pool(name="big", bufs=1) as big,
            tc.tile_pool(name="work", bufs=4) as work,
            tc.tile_pool(name="ps", bufs=1, space="PSUM") as ps,
            tc.tile_pool(name="dram", bufs=1, space="DRAM") as dpool,
        ):
            palls = [
                dpool.tile([128 * NW, R], f32, name=f"pall{t}") for t in range(TC)
            ]

            def body(_iv=None):
                # ---- resident loads: xt pieces on the SP HWDGE ring, ac
                # pieces on the ACT ring so the two input streams issue in
                # parallel instead of serializing on one queue ----
                xts, acs = [], []
                for h in range(halves):
                    xts.append(
                        big.tile([128, KC, BATCH], din, tag=f"xt{h}", name=f"xt{h}")
                    )
                    acs.append(
                        big.tile([128, KC, NFREE], din, tag=f"ac{h}", name=f"ac{h}")
                    )
                off = 0
                for npc in PIECES:
                    sl = slice(off, off + npc)
                    for h in range(halves):
                        nc.sync.dma_start(
                            xts[h][:, sl, :],
                            xt_d[h].rearrange("(kc p) t -> p kc t", p=128)[:, sl, :],
                        )
                        nc.scalar.dma_start(
                            acs[h][:, sl, :],
                            ac_d[h].rearrange("kc p n -> p kc n")[:, sl, :],
                        )
                    off += npc

                g = big.tile([128, nchunk, R], f32, name="g")
                tmp = big.tile([128, nchunk, R], f32, name="tmp")
                outsb = big.tile([128, nchunk], f32, name="outsb")

                # ---- main matmul: kc pieces interleaved across all tc so the
                # PE never waits on the tail of the input stream ----
                psts = [
                    [
                        ps.tile(
                            [128, hi - lo], f32, tag=f"ps{t}_{ci}", name=f"ps{t}_{ci}"
                        )
                        for ci, (lo, hi) in enumerate(chunks)
                    ]
                    for t in range(TC)
                ]
                def emit_mms(t, kcs):
                    for kc in kcs:
                        lhs = [xt[:, kc, t * 128 : (t + 1) * 128] for xt in xts]
                        rhs = [ac[:, kc, :] for ac in acs]
                        for ci, (lo, hi) in enumerate(chunks):
                            if halves == 1:
                                nc.tensor.matmul(
                                    psts[t][ci][:],
                                    lhs[0],
                                    rhs[0][:, lo:hi],
                                    start=(kc == 0),
                                    stop=(kc == KC - 1),
                                )
                            else:  # split: hi*hi + hi*lo + lo*hi
                                for mi, (la, rb) in enumerate(((0, 0), (0, 1), (1, 0))):
                                    nc.tensor.matmul(
                                        psts[t][ci][:],
                                        lhs[la],
                                        rhs[rb][:, lo:hi],
                                        start=(kc == 0 and mi == 0),
                                        stop=(kc == KC - 1 and mi == 2),
                                    )

                # phase 1: kc pieces interleaved across all tc while the input
                # stream lands; phase 2: per-tc sequential so each tc finishes
                # early and its extraction overlaps the remaining matmuls
                off = 0
                for npc in PIECES:
                    if off >= SPLIT_KC:
                        break
                    for t in range(TC):
                        emit_mms(t, range(off, off + npc))
                    off += npc

                gidx = big.tile([128, sum(gcols)], mybir.dt.int16, name="gidx")
                nc.sync.dma_start(gidx[:], gidx_d[:])
                msk = big.tile([128, nchunk, R], mdt, name="msk")
                nc.scalar.dma_start(msk[:], mask_d[:])

                # ---- per-tc tail: mm -> P dump -> gather -> mask mul + reduce ----
                # largest gather groups first: the serial end-of-kernel chain
                # then belongs to the smallest group
                goffs = [sum(gchunks[:t]) for t in range(TC)]
                order = sorted(range(TC), key=lambda t: -gchunks[t])
                for t in order:
                    goff = goffs[t]
                    emit_mms(t, range(off, KC))
                    pcopy = work.tile([128, NFREE], f32, tag="pcopy", name="pcopy")
                    for ci, (lo, hi) in enumerate(chunks):
                        # alternate PSUM->SBUF copies between DVE and ACT so
                        # the chunk copies run in parallel
                        eng = nc.vector if ci % 2 == 0 else nc.scalar
                        if ci % 2 == 0:
                            eng.tensor_copy(pcopy[:, lo:hi], psts[t][ci][:])
                        else:
                            eng.copy(pcopy[:, lo:hi], psts[t][ci][:])
                    # pall dump on the SWDGE path: keeps the SP/ACT HWDGE
                    # rings free of data-dependent stalls so the next rep's
                    # input stream issues while this rep's tail drains
                    nc.gpsimd.dma_start(
                        palls[t][:].rearrange("(p w) r -> p (w r)", p=128), pcopy[:]
                    )
                    coff = sum(gcols[:t])
                    nidx = gchunks[t] * 128
                    nc.gpsimd.dma_gather(
                        out_ap=g[:, goff : goff + gchunks[t], :],
                        in_ap=palls[t][:],
                        idxs_ap=gidx[:, coff : coff + gcols[t]],
                        num_idxs=nidx,
                        num_idxs_reg=nidx,
                        elem_size=R,
                        queue_num=0,
                    )
                    gsl = slice(goff, goff + gchunks[t])
                    nc.vector.tensor_tensor(
                        out=tmp[:, gsl, :],
                        in0=g[:, gsl, :],
                        in1=msk[:, gsl, :],
                        op=mybir.AluOpType.mult,
                    )
                    nc.vector.tensor_reduce(
                        out=outsb[:, gsl],
                        in_=tmp[:, gsl, :],
                        axis=mybir.AxisListType.X,
                        op=mybir.AluOpType.add,
                    )
                    nc.gpsimd.dma_start(out_d[:, gsl], outsb[:, gsl])

            if REPS > 1:
                with tc.For_i(0, REPS, 1):
                    body()
            else:
                body()

    nc.compile()
    return nc


def _split_bf16(a: np.ndarray):
    hi = a.astype(ml_dtypes.bfloat16)
    lo = (a - hi.astype(np.float32)).astype(ml_dtypes.bfloat16)
    return hi, lo


def prepare(x, xids, wids, A):
    """Host-side planning + per-core input buffers. Returns (nc, in_maps, meta)."""
    x = np.ascontiguousarray(np.asarray(x).reshape(BATCH, K), dtype=np.float32)
    xids = np.asarray(xids).astype(np.int64)
    wids = np.asarray(wids).astype(np.int64)
    A = np.ascontiguousarray(np.asarray(A), dtype=np.float32)

    core_slots = _plan(wids)
    tok = xids.reshape(C, R)

    # per-core (c, r, slot) lists grouped by token chunk, padded per group
    core_groups = []  # [core][t] -> list of (c, r, sl)
    for core in range(NCORES):
        groups = [[] for _ in range(TC)]
        for sl, (w, rows) in enumerate(core_slots[core]):
            for c in rows:
                for r in range(R):
                    groups[tok[c, r] // 128].append((c, r, sl))
        core_groups.append(groups)
    gchunks = [
        max(1, max(-(-len(core_groups[core][t]) // 128) for core in range(NCORES)))
        for t in range(TC)
    ]
    nchunk = sum(gchunks)

    xt_f32 = np.ascontiguousarray(x.T)  # [K, BATCH]

    in_maps, pair_lists = [], []
    for core in range(NCORES):
        ws = [w for w, _ in core_slots[core]]
        acore = A[ws]  # [NW, K, R] (duplicates for split adapters)
        ac_f32 = np.ascontiguousarray(acore.transpose(1, 0, 2).reshape(KC, 128, NFREE))

        idx = np.zeros(nchunk * 128, dtype=np.int64)
        rvals = np.zeros(nchunk * 128, dtype=np.int64)
        pairs = np.full((nchunk * 128, 2), -1, dtype=np.int64)
        goff = 0
        gcol_blocks = []
        for t in range(TC):
            for i, (c, r, sl) in enumerate(core_groups[core][t]):
                m = goff * 128 + i
                tt = tok[c, r]
                idx[m] = (tt % 128) * NW + sl
                rvals[m] = r
                pairs[m] = (c, r)
            # wrapped int16 layout for this group: [16, nidx/16] -> tile rows
            nidx = gchunks[t] * 128
            gi = idx[goff * 128 : goff * 128 + nidx]
            blk = np.zeros((16, nidx // 16), dtype=np.int16)
            blk[np.arange(nidx) % 16, np.arange(nidx) // 16] = gi
            gcol_blocks.append(np.tile(blk, (8, 1)))  # replicate to 128 rows
            goff += gchunks[t]
        gidx = np.ascontiguousarray(np.concatenate(gcol_blocks, axis=1))
        mdt = ml_dtypes.bfloat16 if MASK_BF16 else np.float32
        mask = np.zeros((128, nchunk, R), dtype=mdt)
        rv = rvals.reshape(nchunk, 128).T  # [128, nchunk]
        p_i, j_i = np.meshgrid(np.arange(128), np.arange(nchunk), indexing="ij")
        mask[p_i, j_i, rv] = 1.0

        m = {"gidx": gidx, "mask": mask}
        if MODE in ("float32", "float32r"):
            m["xt0"], m["ac0"] = xt_f32, ac_f32
        elif MODE == "bf16":
            m["xt0"] = xt_f32.astype(ml_dtypes.bfloat16)
            m["ac0"] = ac_f32.astype(ml_dtypes.bfloat16)
        else:  # split
            m["xt0"], m["xt1"] = _split_bf16(xt_f32)
            m["ac0"], m["ac1"] = _split_bf16(ac_f32)
        in_maps.append(m)
        pair_lists.append(pairs)

    nc = _build_bass(gchunks)
    return nc, in_maps, (pair_lists, nchunk)


def finish(results, meta):
    pair_lists, nchunk = meta
    out = np.zeros((C, 1, R), dtype=np.float32)
    for core in range(NCORES):
        vals = np.asarray(results[core]["out"]).T.reshape(-1)  # m = j*128+p
        pairs = pair_lists[core]
        sel = pairs[:, 0] >= 0
        out[pairs[sel, 0], 0, pairs[sel, 1]] = vals[sel]
    return out


def kernel(x, xids, wids, A):
    nc, in_maps, meta = prepare(x, xids, wids, A)
    res = run_bass_kernel_spmd(nc, in_maps, core_ids=list(range(NCORES)))
    return finish(res.results, meta)


if __name__ == "__main__":
    rng = np.random.default_rng(0)
    x = rng.standard_normal((BATCH, 1, K), dtype=np.float32)
    xids = rng.integers(0, BATCH, C * R).astype(np.int32)
    wids = rng.integers(0, NA, C).astype(np.int32)
    A = (rng.standard_normal((NA, K, R)) * 0.02).astype(np.float32)
    got = kernel(x=x, xids=xids, wids=wids, A=A)
    tokh = xids.reshape(C, R)
    want = np.einsum(
        "crk,ckr->cr",
        x[tokh, 0, :].astype(np.float64),
        A[wids].astype(np.float64),
    )[:, None, :]
    rel = np.abs(got - want).max() / np.abs(want).max()
    print(f"MODE={MODE} rel err vs f64: {rel:.3e}")



# revision 17
# speedup vs baseline: 1.0453x; 1.0453x over previous
"""TRN2 Bass kernel for nn_CombinedLoraA (moe_routing).

Computation: out[c, 0, r] = sum_k x[xids[c*64+r], 0, k] * A[wids[c], k, r]
  x: (512, 1, 4096) f32, xids: (20480,) i32, wids: (320,) i32, A: (80, 4096, 64) f32
  out: (320, 1, 64) f32

Strategy (adapter-parallel across 8 cores, routing baked in at trace time):
  - Host assigns exactly 10 adapters to each core (greedy row-count balance).
  - Each core computes the dense precompute P[w, t, r] = sum_k X[t, k] A[w, k, r]
    for ALL 512 tokens x its 10 adapters on the PE (X^T stationary, adapters'
    columns concatenated in the moving operand). X^T is transposed on the host.
  - P is copied PSUM->SBUF->DRAM per 128-token chunk (4 separate DRAM tables
    so extraction for chunk t overlaps chunk t+1's matmuls); the needed
    out[c, r] = P[w_c, tok[c, r], r] elements are fetched with one dma_gather
    per chunk (64-f32 rows, int16 indices), then a host-baked one-hot mask
    multiply + free-axis reduce picks the right column per (c, r) pair.
  - Host stitches the 8 per-core staging buffers into the (320, 1, 64) output.

Precision modes for the big matmul (error measured on HW at K=4096):
  float32 (3e-7), split bf16 hi/lo 3-matmul (4.5e-6), float32r (1.7e-4),
  bf16 (2.4e-3).
"""

import os
import sys

sys.path.insert(0, "/opt/trn_rl_repo")

import numpy as np
import ml_dtypes

import concourse.tile as tile
from concourse import mybir, bacc
from concourse.bass_utils import run_bass_kernel_spmd

BATCH, C, R, K, NA = 512, 320, 64, 4096, 80
NCORES = 8
NW = NA // NCORES  # 10 adapters per core
KC = K // 128  # 32 contraction chunks
TC = BATCH // 128  # 4 token chunks
NFREE = NW * R  # 640 moving columns in the main matmul
PIECES = [1, 1] + [2] * 15  # kc-sized DMA pieces (sum = KC)

SPLIT_KC = int(os.environ.get("SPLIT_KC", "20"))
MODE = os.environ.get("KERNEL_MODE", "bf16")  # float32 | split | float32r | bf16
REPS = int(os.environ.get("BENCH_REPS", "1"))
MASK_BF16 = os.environ.get("MASK_BF16", "1") == "1"
# tok: x^T stationary, A-columns moving (4 token-block groups, 256x320-col MMs)
# col: A-columns stationary, tokens moving (5 col-block groups, 160x512-col MMs)
ORIENT = os.environ.get("ORIENT", "tok")
NCB = NW // 2  # col-blocks per core (2 slots x 64 r = 128 stationary cols each)


def _plan(wids: np.ndarray):
    """Assign exactly NW (adapter, row-sublist) slots per core, balancing row
    count. An adapter with more than ROWCAP rows is split across slots (its
    weights are simply duplicated in the per-core A buffer), which bounds any
    core's row count and therefore the extraction buffer sizes even for
    degenerate wids distributions. For uniform wids (max rows/adapter well
    under ROWCAP) this reduces to one slot per adapter."""
    ROWCAP = 40
    rows_of = [[] for _ in range(NA)]
    for c, w in enumerate(wids):
        rows_of[w].append(c)
    slots = []  # (w, row sublist)
    for w in range(NA):
        rows = rows_of[w]
        if not rows:
            slots.append((w, []))
        else:
            for i in range(0, len(rows), ROWCAP):
                slots.append((w, rows[i : i + ROWCAP]))
    # keep exactly NCORES*NW slots: drop empty ones if over, pad if under
    empty = [s for s in slots if not s[1]]
    full = [s for s in slots if s[1]]
    assert len(full) <= NCORES * NW, "too many hot adapters to split"
    slots = full + empty[: NCORES * NW - len(full)]
    while len(slots) < NCORES * NW:
        slots.append((0, []))
    order = sorted(range(len(slots)), key=lambda s: -len(slots[s][1]))
    core_slots = [[] for _ in range(NCORES)]
    core_load = [0] * NCORES
    for s in order:
        cands = [i for i in range(NCORES) if len(core_slots[i]) < NW]
        i = min(cands, key=lambda j: core_load[j])
        core_slots[i].append(slots[s])
        core_load[i] += len(slots[s][1])
    return core_slots


def _chunks():
    n = int(os.environ.get("NCHUNKS", "2"))
    w = NFREE // n
    return tuple((i * w, (i + 1) * w) for i in range(n))


def _mode_cfg():
    f32, bf16 = mybir.dt.float32, mybir.dt.bfloat16
    if MODE == "float32":
        return f32, 1, _chunks()
    if MODE == "float32r":
        return mybir.dt.float32r, 1, _chunks()
    if MODE == "bf16":
        return bf16, 1, _chunks()
    return bf16, 2, _chunks()  # split


def _build_bass(gchunks: list[int]):
    """gchunks[t] = number of 128-row gather chunks for token-chunk group t."""
    nc = bacc.Bacc("TRN2", target_bir_lowering=False, num_swdge_queues=2)
    f32 = mybir.dt.float32
    mdt = mybir.dt.bfloat16 if MASK_BF16 else f32
    din, halves, chunks = _mode_cfg()
    nchunk = sum(gchunks)

    xt_d = [
        nc.dram_tensor(f"xt{h}", [K, BATCH], din, kind="ExternalInput")
        for h in range(halves)
    ]
    ac_d = [
        nc.dram_tensor(f"ac{h}", [KC, 128, NFREE], din, kind="ExternalInput")
        for h in range(halves)
    ]
    gcols = [gc * 8 for gc in gchunks]  # idx columns per group (num_idxs/16)
    gidx_d = nc.dram_tensor(
        "gidx", [128, sum(gcols)], mybir.dt.int16, kind="ExternalInput"
    )
    mask_d = nc.dram_tensor("mask", [128, nchunk, R], mdt, kind="ExternalInput")
    out_d = nc.dram_tensor("out", [128, nchunk], f32, kind="ExternalOutput")

    with tile.TileContext(nc) as tc:
        with (
            tc.tile_pool(name="big", bufs=1) as big,
            tc.tile_pool(name="work", bufs=4) as work,
            tc.tile_pool(name="ps", bufs=1, space="PSUM") as ps,
            tc.tile_pool(name="dram", bufs=1, space="DRAM") as dpool,
        ):
            palls = [
                dpool.tile([128 * NW, R], f32, name=f"pall{t}") for t in range(TC)
            ]

            def body(_iv=None):
                # ---- resident loads: xt pieces on the SP HWDGE ring, ac
                # pieces on the ACT ring so the two input streams issue in
                # parallel instead of serializing on one queue ----
                xts, acs = [], []
                for h in range(halves):
                    xts.append(
                        big.tile([128, KC, BATCH], din, tag=f"xt{h}", name=f"xt{h}")
                    )
                    acs.append(
                        big.tile([128, KC, NFREE], din, tag=f"ac{h}", name=f"ac{h}")
                    )
                off = 0
                for npc in PIECES:
                    sl = slice(off, off + npc)
                    for h in range(halves):
                        nc.sync.dma_start(
                            xts[h][:, sl, :],
                            xt_d[h].rearrange("(kc p) t -> p kc t", p=128)[:, sl, :],
                        )
                        nc.scalar.dma_start(
                            acs[h][:, sl, :],
                            ac_d[h].rearrange("kc p n -> p kc n")[:, sl, :],
                        )
                    off += npc

                g = big.tile([128, nchunk, R], f32, name="g")
                tmp = big.tile([128, nchunk, R], f32, name="tmp")
                outsb = big.tile([128, nchunk], f32, name="outsb")

                # ---- main matmul: kc pieces interleaved across all tc so the
                # PE never waits on the tail of the input stream ----
                psts = [
                    [
                        ps.tile(
                            [128, hi - lo], f32, tag=f"ps{t}_{ci}", name=f"ps{t}_{ci}"
                        )
                        for ci, (lo, hi) in enumerate(chunks)
                    ]
                    for t in range(TC)
                ]
                def emit_mms(t, kcs):
                    for kc in kcs:
                        lhs = [xt[:, kc, t * 128 : (t + 1) * 128] for xt in xts]
                        rhs = [ac[:, kc, :] for ac in acs]
                        for ci, (lo, hi) in enumerate(chunks):
                            if halves == 1:
                                nc.tensor.matmul(
                                    psts[t][ci][:],
                                    lhs[0],
                                    rhs[0][:, lo:hi],
                                    start=(kc == 0),
                                    stop=(kc == KC - 1),
                                )
                            else:  # split: hi*hi + hi*lo + lo*hi
                                for mi, (la, rb) in enumerate(((0, 0), (0, 1), (1, 0))):
                                    nc.tensor.matmul(
                                        psts[t][ci][:],
                                        lhs[la],
                                        rhs[rb][:, lo:hi],
                                        start=(kc == 0 and mi == 0),
                                        stop=(kc == KC - 1 and mi == 2),
                                    )

                # phase 1: kc pieces interleaved across all tc while the input
                # stream lands; phase 2: per-tc sequential so each tc finishes
                # early and its extraction overlaps the remaining matmuls
                off = 0
                for npc in PIECES:
                    if off >= SPLIT_KC:
                        break
                    for t in range(TC):
                        emit_mms(t, range(off, off + npc))
                    off += npc

                gidx = big.tile([128, sum(gcols)], mybir.dt.int16, name="gidx")
                nc.sync.dma_start(gidx[:], gidx_d[:])
                msk = big.tile([128, nchunk, R], mdt, name="msk")
                nc.scalar.dma_start(msk[:], mask_d[:])

                # ---- per-tc tail: mm -> P dump -> gather -> mask mul + reduce ----
                # largest gather groups first: the serial end-of-kernel chain
                # then belongs to the smallest group
                goffs = [sum(gchunks[:t]) for t in range(TC)]
                order = sorted(range(TC), key=lambda t: -gchunks[t])
                for t in order:
                    goff = goffs[t]
                    emit_mms(t, range(off, KC))
                    pcopy = work.tile([128, NFREE], f32, tag="pcopy", name="pcopy")
                    for ci, (lo, hi) in enumerate(chunks):
                        # alternate PSUM->SBUF copies between DVE and ACT so
                        # the chunk copies run in parallel
                        eng = nc.vector if ci % 2 == 0 else nc.scalar
                        if ci % 2 == 0:
                            eng.tensor_copy(pcopy[:, lo:hi], psts[t][ci][:])
                        else:
                            eng.copy(pcopy[:, lo:hi], psts[t][ci][:])
                    nc.sync.dma_start(
                        palls[t][:].rearrange("(p w) r -> p (w r)", p=128), pcopy[:]
                    )
                    coff = sum(gcols[:t])
                    nidx = gchunks[t] * 128
                    nc.gpsimd.dma_gather(
                        out_ap=g[:, goff : goff + gchunks[t], :],
                        in_ap=palls[t][:],
                        idxs_ap=gidx[:, coff : coff + gcols[t]],
                        num_idxs=nidx,
                        num_idxs_reg=nidx,
                        elem_size=R,
                        queue_num=t % 2,
                    )
                    gsl = slice(goff, goff + gchunks[t])
                    nc.vector.tensor_tensor(
                        out=tmp[:, gsl, :],
                        in0=g[:, gsl, :],
                        in1=msk[:, gsl, :],
                        op=mybir.AluOpType.mult,
                    )
                    nc.vector.tensor_reduce(
                        out=outsb[:, gsl],
                        in_=tmp[:, gsl, :],
                        axis=mybir.AxisListType.X,
                        op=mybir.AluOpType.add,
                    )
                    nc.scalar.dma_start(out_d[:, gsl], outsb[:, gsl])

            if REPS > 1:
                with tc.For_i(0, REPS, 1):
                    body()
            else:
                body()

    nc.compile()
    return nc


def _split_bf16(a: np.ndarray):
    hi = a.astype(ml_dtypes.bfloat16)
    lo = (a - hi.astype(np.float32)).astype(ml_dtypes.bfloat16)
    return hi, lo


def prepare(x, xids, wids, A):
    """Host-side planning + per-core input buffers. Returns (nc, in_maps, meta)."""
    x = np.ascontiguousarray(np.asarray(x).reshape(BATCH, K), dtype=np.float32)
    xids = np.asarray(xids).astype(np.int64)
    wids = np.asarray(wids).astype(np.int64)
    A = np.ascontiguousarray(np.asarray(A), dtype=np.float32)

    core_slots = _plan(wids)
    tok = xids.reshape(C, R)

    # per-core (c, r, slot) lists grouped by token chunk, padded per group
    core_groups = []  # [core][t] -> list of (c, r, sl)
    for core in range(NCORES):
        groups = [[] for _ in range(TC)]
        for sl, (w, rows) in enumerate(core_slots[core]):
            for c in rows:
                for r in range(R):
                    groups[tok[c, r] // 128].append((c, r, sl))
        core_groups.append(groups)
    gchunks = [
        max(1, max(-(-len(core_groups[core][t]) // 128) for core in range(NCORES)))
        for t in range(TC)
    ]
    nchunk = sum(gchunks)

    xt_f32 = np.ascontiguousarray(x.T)  # [K, BATCH]

    in_maps, pair_lists = [], []
    for core in range(NCORES):
        ws = [w for w, _ in core_slots[core]]
        acore = A[ws]  # [NW, K, R] (duplicates for split adapters)
        ac_f32 = np.ascontiguousarray(acore.transpose(1, 0, 2).reshape(KC, 128, NFREE))

        idx = np.zeros(nchunk * 128, dtype=np.int64)
        rvals = np.zeros(nchunk * 128, dtype=np.int64)
        pairs = np.full((nchunk * 128, 2), -1, dtype=np.int64)
        goff = 0
        gcol_blocks = []
        for t in range(TC):
            for i, (c, r, sl) in enumerate(core_groups[core][t]):
                m = goff * 128 + i
                tt = tok[c, r]
                idx[m] = (tt % 128) * NW + sl
                rvals[m] = r
                pairs[m] = (c, r)
            # wrapped int16 layout for this group: [16, nidx/16] -> tile rows
            nidx = gchunks[t] * 128
            gi = idx[goff * 128 : goff * 128 + nidx]
            blk = np.zeros((16, nidx // 16), dtype=np.int16)
            blk[np.arange(nidx) % 16, np.arange(nidx) // 16] = gi
            gcol_blocks.append(np.tile(blk, (8, 1)))  # replicate to 128 rows
            goff += gchunks[t]
        gidx = np.ascontiguousarray(np.concatenate(gcol_blocks, axis=1))
        mdt = ml_dtypes.bfloat16 if MASK_BF16 else np.float32
        mask = np.zeros((128, nchunk, R), dtype=mdt)
        rv = rvals.reshape(nchunk, 128).T  # [128, nchunk]
        p_i, j_i = np.meshgrid(np.arange(128), np.arange(nchunk), indexing="ij")
        mask[p_i, j_i, rv] = 1.0

        m = {"gidx": gidx, "mask": mask}
        if MODE in ("float32", "float32r"):
            m["xt0"], m["ac0"] = xt_f32, ac_f32
        elif MODE == "bf16":
            m["xt0"] = xt_f32.astype(ml_dtypes.bfloat16)
            m["ac0"] = ac_f32.astype(ml_dtypes.bfloat16)
        else:  # split
            m["xt0"], m["xt1"] = _split_bf16(xt_f32)
            m["ac0"], m["ac1"] = _split_bf16(ac_f32)
        in_maps.append(m)
        pair_lists.append(pairs)

    nc = _build_bass(gchunks)
    return nc, in_maps, (pair_lists, nchunk)


def finish(results, meta):
    pair_lists, nchunk = meta
    out = np.zeros((C, 1, R), dtype=np.float32)
    for core in range(NCORES):
        vals = np.asarray(results[core]["out"]).T.reshape(-1)  # m = j*128+p
        pairs = pair_lists[core]
        sel = pairs[:, 0] >= 0
        out[pairs[sel, 0], 0, pairs[sel, 1]] = vals[sel]
    return out


def kernel(x, xids, wids, A):
    nc, in_maps, meta = prepare(x, xids, wids, A)
    res = run_bass_kernel_spmd(nc, in_maps, core_ids=list(range(NCORES)))
    return finish(res.results, meta)


if __name__ == "__main__":
    rng = np.random.default_rng(0)
    x = rng.standard_normal((BATCH, 1, K), dtype=np.float32)
    xids = rng.integers(0, BATCH, C * R).astype(np.int32)
    wids = rng.integers(0, NA, C).astype(np.int32)
    A = (rng.standard_normal((NA, K, R)) * 0.02).astype(np.float32)
    got = kernel(x=x, xids=xids, wids=wids, A=A)
    tokh = xids.reshape(C, R)
    want = np.einsum(
        "crk,ckr->cr",
        x[tokh, 0, :].astype(np.float64),
        A[wids].astype(np.float64),
    )[:, None, :]
    rel = np.abs(got - want).max() / np.abs(want).max()
    print(f"MODE={MODE} rel err vs f64: {rel:.3e}")



# revision 21
# speedup vs baseline: 1.2949x; 1.2387x over previous
"""TRN2 Bass kernel for nn_CombinedLoraA (moe_routing).

Computation: out[c, 0, r] = sum_k x[xids[c*64+r], 0, k] * A[wids[c], k, r]
  x: (512, 1, 4096) f32, xids: (20480,) i32, wids: (320,) i32, A: (80, 4096, 64) f32
  out: (320, 1, 64) f32

Strategy (adapter-parallel across 8 cores, routing baked in at trace time):
  - Host assigns exactly 10 adapters to each core (greedy row-count balance).
  - Each core computes the dense precompute P[w, t, r] = sum_k X[t, k] A[w, k, r]
    for ALL 512 tokens x its 10 adapters on the PE (X^T stationary, adapters'
    columns concatenated in the moving operand). X^T is transposed on the host.
  - P is copied PSUM->SBUF->DRAM per 128-token chunk (4 separate DRAM tables
    so extraction for chunk t overlaps chunk t+1's matmuls); the needed
    out[c, r] = P[w_c, tok[c, r], r] elements are fetched with one dma_gather
    per chunk (64-f32 rows, int16 indices), then a host-baked one-hot mask
    multiply + free-axis reduce picks the right column per (c, r) pair.
  - Host stitches the 8 per-core staging buffers into the (320, 1, 64) output.

Precision modes for the big matmul (error measured on HW at K=4096):
  float32 (3e-7), split bf16 hi/lo 3-matmul (4.5e-6), float32r (1.7e-4),
  bf16 (2.4e-3).
"""

import os
import sys

sys.path.insert(0, "/opt/trn_rl_repo")

import numpy as np
import ml_dtypes

import concourse.tile as tile
from concourse import mybir, bacc
from concourse.bass_utils import run_bass_kernel_spmd

BATCH, C, R, K, NA = 512, 320, 64, 4096, 80
NCORES = 8
NW = NA // NCORES  # 10 adapters per core
KC = K // 128  # 32 contraction chunks
TC = BATCH // 128  # 4 token chunks
NFREE = NW * R  # 640 moving columns in the main matmul
PIECES = [1, 1] + [2] * 15  # kc-sized DMA pieces (sum = KC)

SPLIT_KC = int(os.environ.get("SPLIT_KC", "20"))
MODE = os.environ.get("KERNEL_MODE", "bf16")  # float32 | split | float32r | bf16
REPS = int(os.environ.get("BENCH_REPS", "1"))
MASK_BF16 = os.environ.get("MASK_BF16", "1") == "1"
# tok: x^T stationary, A-columns moving (4 token-block groups, 256x320-col MMs)
# col: A-columns stationary, tokens moving (5 col-block groups, 160x512-col MMs)
ORIENT = os.environ.get("ORIENT", "tok")
NCB = NW // 2  # col-blocks per core (2 slots x 64 r = 128 stationary cols each)
WARM_MMS = int(os.environ.get("WARM_MMS", "0"))  # PE warm-keeper dummies in tail
FINE_TAIL = os.environ.get("FINE_TAIL", "0") == "1"  # split last-group extraction


def _plan(wids: np.ndarray):
    """Assign exactly NW (adapter, row-sublist) slots per core, balancing row
    count. An adapter with more than ROWCAP rows is split across slots (its
    weights are simply duplicated in the per-core A buffer), which bounds any
    core's row count and therefore the extraction buffer sizes even for
    degenerate wids distributions. For uniform wids (max rows/adapter well
    under ROWCAP) this reduces to one slot per adapter."""
    ROWCAP = 40
    rows_of = [[] for _ in range(NA)]
    for c, w in enumerate(wids):
        rows_of[w].append(c)
    slots = []  # (w, row sublist)
    for w in range(NA):
        rows = rows_of[w]
        if not rows:
            slots.append((w, []))
        else:
            for i in range(0, len(rows), ROWCAP):
                slots.append((w, rows[i : i + ROWCAP]))
    # keep exactly NCORES*NW slots: drop empty ones if over, pad if under
    empty = [s for s in slots if not s[1]]
    full = [s for s in slots if s[1]]
    assert len(full) <= NCORES * NW, "too many hot adapters to split"
    slots = full + empty[: NCORES * NW - len(full)]
    while len(slots) < NCORES * NW:
        slots.append((0, []))
    order = sorted(range(len(slots)), key=lambda s: -len(slots[s][1]))
    core_slots = [[] for _ in range(NCORES)]
    core_load = [0] * NCORES
    for s in order:
        cands = [i for i in range(NCORES) if len(core_slots[i]) < NW]
        i = min(cands, key=lambda j: core_load[j])
        core_slots[i].append(slots[s])
        core_load[i] += len(slots[s][1])
    return core_slots


def _chunks():
    n = int(os.environ.get("NCHUNKS", "2"))
    w = NFREE // n
    return tuple((i * w, (i + 1) * w) for i in range(n))


def _mode_cfg():
    f32, bf16 = mybir.dt.float32, mybir.dt.bfloat16
    if MODE == "float32":
        return f32, 1, _chunks()
    if MODE == "float32r":
        return mybir.dt.float32r, 1, _chunks()
    if MODE == "bf16":
        return bf16, 1, _chunks()
    return bf16, 2, _chunks()  # split


def _build_bass(gchunks: list[int]):
    """gchunks[t] = number of 128-row gather chunks for token-chunk group t."""
    nc = bacc.Bacc("TRN2", target_bir_lowering=False, num_swdge_queues=2)
    f32 = mybir.dt.float32
    mdt = mybir.dt.bfloat16 if MASK_BF16 else f32
    din, halves, chunks = _mode_cfg()
    nchunk = sum(gchunks)

    xt_d = [
        nc.dram_tensor(f"xt{h}", [K, BATCH], din, kind="ExternalInput")
        for h in range(halves)
    ]
    ac_d = [
        nc.dram_tensor(f"ac{h}", [KC, 128, NFREE], din, kind="ExternalInput")
        for h in range(halves)
    ]
    gcols = [gc * 8 for gc in gchunks]  # idx columns per group (num_idxs/16)
    gidx_d = nc.dram_tensor(
        "gidx", [128, sum(gcols)], mybir.dt.int16, kind="ExternalInput"
    )
    mask_d = nc.dram_tensor("mask", [128, nchunk, R], mdt, kind="ExternalInput")
    out_d = nc.dram_tensor("out", [128, nchunk], f32, kind="ExternalOutput")

    with tile.TileContext(nc) as tc:
        with (
            tc.tile_pool(name="big", bufs=1) as big,
            tc.tile_pool(name="work", bufs=4) as work,
            tc.tile_pool(name="ps", bufs=1, space="PSUM") as ps,
            tc.tile_pool(name="dram", bufs=1, space="DRAM") as dpool,
        ):
            palls = [
                dpool.tile([128 * NW, R], f32, name=f"pall{t}") for t in range(TC)
            ]

            def body(_iv=None):
                # ---- resident loads: xt pieces on the SP HWDGE ring, ac
                # pieces on the ACT ring so the two input streams issue in
                # parallel instead of serializing on one queue ----
                xts, acs = [], []
                for h in range(halves):
                    xts.append(
                        big.tile([128, KC, BATCH], din, tag=f"xt{h}", name=f"xt{h}")
                    )
                    acs.append(
                        big.tile([128, KC, NFREE], din, tag=f"ac{h}", name=f"ac{h}")
                    )
                off = 0
                for npc in PIECES:
                    sl = slice(off, off + npc)
                    for h in range(halves):
                        nc.sync.dma_start(
                            xts[h][:, sl, :],
                            xt_d[h].rearrange("(kc p) t -> p kc t", p=128)[:, sl, :],
                        )
                        nc.scalar.dma_start(
                            acs[h][:, sl, :],
                            ac_d[h].rearrange("kc p n -> p kc n")[:, sl, :],
                        )
                    off += npc

                g = big.tile([128, nchunk, R], f32, name="g")
                tmp = big.tile([128, nchunk, R], f32, name="tmp")
                outsb = big.tile([128, nchunk], f32, name="outsb")

                # ---- main matmul: kc pieces interleaved across all tc so the
                # PE never waits on the tail of the input stream ----
                psts = [
                    [
                        ps.tile(
                            [128, hi - lo], f32, tag=f"ps{t}_{ci}", name=f"ps{t}_{ci}"
                        )
                        for ci, (lo, hi) in enumerate(chunks)
                    ]
                    for t in range(TC)
                ]
                def emit_mms(t, kcs):
                    for kc in kcs:
                        lhs = [xt[:, kc, t * 128 : (t + 1) * 128] for xt in xts]
                        rhs = [ac[:, kc, :] for ac in acs]
                        for ci, (lo, hi) in enumerate(chunks):
                            if halves == 1:
                                nc.tensor.matmul(
                                    psts[t][ci][:],
                                    lhs[0],
                                    rhs[0][:, lo:hi],
                                    start=(kc == 0),
                                    stop=(kc == KC - 1),
                                )
                            else:  # split: hi*hi + hi*lo + lo*hi
                                for mi, (la, rb) in enumerate(((0, 0), (0, 1), (1, 0))):
                                    nc.tensor.matmul(
                                        psts[t][ci][:],
                                        lhs[la],
                                        rhs[rb][:, lo:hi],
                                        start=(kc == 0 and mi == 0),
                                        stop=(kc == KC - 1 and mi == 2),
                                    )

                # phase 1: kc pieces interleaved across all tc while the input
                # stream lands; phase 2: per-tc sequential so each tc finishes
                # early and its extraction overlaps the remaining matmuls
                off = 0
                for npc in PIECES:
                    if off >= SPLIT_KC:
                        break
                    for t in range(TC):
                        emit_mms(t, range(off, off + npc))
                    off += npc

                gidx = big.tile([128, sum(gcols)], mybir.dt.int16, name="gidx")
                nc.sync.dma_start(gidx[:], gidx_d[:])
                msk = big.tile([128, nchunk, R], mdt, name="msk")
                nc.scalar.dma_start(msk[:], mask_d[:])

                # ---- per-tc tail: mm -> P dump -> gather -> mask mul + reduce ----
                # largest gather groups first: the serial end-of-kernel chain
                # then belongs to the smallest group
                goffs = [sum(gchunks[:t]) for t in range(TC)]
                order = sorted(range(TC), key=lambda t: -gchunks[t])
                for t in order:
                    goff = goffs[t]
                    emit_mms(t, range(off, KC))
                    pcopy = work.tile([128, NFREE], f32, tag="pcopy", name="pcopy")
                    for ci, (lo, hi) in enumerate(chunks):
                        nc.vector.tensor_copy(pcopy[:, lo:hi], psts[t][ci][:])
                    nc.sync.dma_start(
                        palls[t][:].rearrange("(p w) r -> p (w r)", p=128), pcopy[:]
                    )
                    coff = sum(gcols[:t])
                    nidx = gchunks[t] * 128
                    nc.gpsimd.dma_gather(
                        out_ap=g[:, goff : goff + gchunks[t], :],
                        in_ap=palls[t][:],
                        idxs_ap=gidx[:, coff : coff + gcols[t]],
                        num_idxs=nidx,
                        num_idxs_reg=nidx,
                        elem_size=R,
                        queue_num=t % 2,
                    )
                    gsl = slice(goff, goff + gchunks[t])
                    nc.vector.tensor_tensor(
                        out=tmp[:, gsl, :],
                        in0=g[:, gsl, :],
                        in1=msk[:, gsl, :],
                        op=mybir.AluOpType.mult,
                    )
                    nc.vector.tensor_reduce(
                        out=outsb[:, gsl],
                        in_=tmp[:, gsl, :],
                        axis=mybir.AxisListType.X,
                        op=mybir.AluOpType.add,
                    )
                    nc.scalar.dma_start(out_d[:, gsl], outsb[:, gsl])

            if REPS > 1:
                with tc.For_i(0, REPS, 1):
                    body()
            else:
                body()

    nc.compile()
    return nc


def _build_bass_col(gchunks: list[int]):
    """Column-stationary orientation: stationary = 128 A-columns (2 slots),
    moving = all 512 tokens. 160 matmuls of 512 cols (vs 256x320): fewer,
    larger PE ops amortize per-instruction overhead. P^T[col, tok] lands in
    5 PSUM banks; extraction gathers 64-token rows and mask-selects tok%64."""
    nc = bacc.Bacc("TRN2", target_bir_lowering=False, num_swdge_queues=2)
    f32 = mybir.dt.float32
    mdt = mybir.dt.bfloat16 if MASK_BF16 else f32
    din, halves, _ = _mode_cfg()
    nchunk = sum(gchunks)
    TB = BATCH // R  # 8 token-blocks of 64 per P^T row

    xt_d = [
        nc.dram_tensor(f"xt{h}", [K, BATCH], din, kind="ExternalInput")
        for h in range(halves)
    ]
    ac_d = [
        nc.dram_tensor(f"ac{h}", [KC, 128, NFREE], din, kind="ExternalInput")
        for h in range(halves)
    ]
    gcols = [gc * 8 for gc in gchunks]
    gidx_d = nc.dram_tensor(
        "gidx", [128, sum(gcols)], mybir.dt.int16, kind="ExternalInput"
    )
    mask_d = nc.dram_tensor("mask", [128, nchunk, R], mdt, kind="ExternalInput")
    out_d = nc.dram_tensor("out", [128, nchunk], f32, kind="ExternalOutput")

    with tile.TileContext(nc) as tc:
        with (
            tc.tile_pool(name="big", bufs=1) as big,
            tc.tile_pool(name="work", bufs=4) as work,
            tc.tile_pool(name="ps", bufs=1, space="PSUM") as ps,
            tc.tile_pool(name="dram", bufs=1, space="DRAM") as dpool,
        ):
            palls = [
                dpool.tile([128 * TB, R], f32, name=f"pall{cb}") for cb in range(NCB)
            ]

            def body(_iv=None):
                xts, acs = [], []
                for h in range(halves):
                    xts.append(
                        big.tile([128, KC, BATCH], din, tag=f"xt{h}", name=f"xt{h}")
                    )
                    acs.append(
                        big.tile([128, KC, NFREE], din, tag=f"ac{h}", name=f"ac{h}")
                    )
                off = 0
                for npc in PIECES:
                    sl = slice(off, off + npc)
                    for h in range(halves):
                        nc.sync.dma_start(
                            xts[h][:, sl, :],
                            xt_d[h].rearrange("(kc p) t -> p kc t", p=128)[:, sl, :],
                        )
                        nc.scalar.dma_start(
                            acs[h][:, sl, :],
                            ac_d[h].rearrange("kc p n -> p kc n")[:, sl, :],
                        )
                    off += npc

                g = big.tile([128, nchunk, R], f32, name="g")
                tmp = big.tile([128, nchunk, R], f32, name="tmp")
                outsb = big.tile([128, nchunk], f32, name="outsb")

                psts = [
                    ps.tile([128, BATCH], f32, tag=f"ps{cb}", name=f"ps{cb}")
                    for cb in range(NCB)
                ]

                def emit_mms(cb, kcs):
                    csl = slice(cb * 128, (cb + 1) * 128)
                    for kc in kcs:
                        if halves == 1:
                            nc.tensor.matmul(
                                psts[cb][:],
                                acs[0][:, kc, csl],
                                xts[0][:, kc, :],
                                start=(kc == 0),
                                stop=(kc == KC - 1),
                            )
                        else:  # split: hi*hi + hi*lo + lo*hi
                            for mi, (a, b) in enumerate(((0, 0), (0, 1), (1, 0))):
                                nc.tensor.matmul(
                                    psts[cb][:],
                                    acs[a][:, kc, csl],
                                    xts[b][:, kc, :],
                                    start=(kc == 0 and mi == 0),
                                    stop=(kc == KC - 1 and mi == 2),
                                )

                off = 0
                for npc in PIECES:
                    if off >= SPLIT_KC:
                        break
                    for cb in range(NCB):
                        emit_mms(cb, range(off, off + npc))
                    off += npc

                gidx = big.tile([128, sum(gcols)], mybir.dt.int16, name="gidx")
                nc.sync.dma_start(gidx[:], gidx_d[:])
                msk = big.tile([128, nchunk, R], mdt, name="msk")
                nc.scalar.dma_start(msk[:], mask_d[:])

                goffs = [sum(gchunks[:cb]) for cb in range(NCB)]
                order = sorted(range(NCB), key=lambda cb: -gchunks[cb])
                for cb in order:
                    goff = goffs[cb]
                    emit_mms(cb, range(off, KC))
                    pcopy = work.tile([128, BATCH], f32, tag="pcopy", name="pcopy")
                    nc.vector.tensor_copy(pcopy[:], psts[cb][:])
                    nc.sync.dma_start(
                        palls[cb][:].rearrange("(p tb) r -> p (tb r)", p=128),
                        pcopy[:],
                    )
                    coff = sum(gcols[:cb])
                    nidx = gchunks[cb] * 128
                    nc.gpsimd.dma_gather(
                        out_ap=g[:, goff : goff + gchunks[cb], :],
                        in_ap=palls[cb][:],
                        idxs_ap=gidx[:, coff : coff + gcols[cb]],
                        num_idxs=nidx,
                        num_idxs_reg=nidx,
                        elem_size=R,
                        queue_num=cb % 2,
                    )
                    gsl = slice(goff, goff + gchunks[cb])
                    nc.vector.tensor_tensor(
                        out=tmp[:, gsl, :],
                        in0=g[:, gsl, :],
                        in1=msk[:, gsl, :],
                        op=mybir.AluOpType.mult,
                    )
                    nc.vector.tensor_reduce(
                        out=outsb[:, gsl],
                        in_=tmp[:, gsl, :],
                        axis=mybir.AxisListType.X,
                        op=mybir.AluOpType.add,
                    )
                    nc.scalar.dma_start(out_d[:, gsl], outsb[:, gsl])

            if REPS > 1:
                with tc.For_i(0, REPS, 1):
                    body()
            else:
                body()

    nc.compile()
    return nc


def _split_bf16(a: np.ndarray):
    hi = a.astype(ml_dtypes.bfloat16)
    lo = (a - hi.astype(np.float32)).astype(ml_dtypes.bfloat16)
    return hi, lo


def prepare_col(x, xids, wids, A):
    """Host planning for ORIENT=col: 5 col-block groups of 2 slots each,
    pair slots to balance staged counts across groups."""
    x = np.ascontiguousarray(np.asarray(x).reshape(BATCH, K), dtype=np.float32)
    xids = np.asarray(xids).astype(np.int64)
    wids = np.asarray(wids).astype(np.int64)
    A = np.ascontiguousarray(np.asarray(A), dtype=np.float32)

    core_slots = _plan(wids)
    tok = xids.reshape(C, R)

    # pair slots into NCB col-blocks, balancing rows per block, then flatten
    # pairs back into the slot order used for the ac buffer
    paired_slots = []
    for core in range(NCORES):
        slots = sorted(core_slots[core], key=lambda s: -len(s[1]))
        blocks = [[] for _ in range(NCB)]
        loads = [0] * NCB
        for s in slots:
            cands = [i for i in range(NCB) if len(blocks[i]) < 2]
            i = min(cands, key=lambda j: loads[j])
            blocks[i].append(s)
            loads[i] += len(s[1])
        for b in blocks:
            while len(b) < 2:
                b.append((0, []))
        paired_slots.append(blocks)

    # per-core staged groups by col-block
    core_groups = []
    for core in range(NCORES):
        groups = [[] for _ in range(NCB)]
        for cb in range(NCB):
            for loc, (w, rows) in enumerate(paired_slots[core][cb]):
                for c in rows:
                    for r in range(R):
                        tt = tok[c, r]
                        colp = loc * R + r
                        groups[cb].append((c, r, colp * 8 + tt // R, tt % R))
        core_groups.append(groups)
    gchunks = [
        max(1, max(-(-len(core_groups[core][cb]) // 128) for core in range(NCORES)))
        for cb in range(NCB)
    ]
    nchunk = sum(gchunks)

    xt_f32 = np.ascontiguousarray(x.T)  # [K, BATCH]

    in_maps, pair_lists = [], []
    for core in range(NCORES):
        ws = [w for cb in range(NCB) for (w, _) in paired_slots[core][cb]]
        acore = A[ws]  # [NW, K, R]
        ac_f32 = np.ascontiguousarray(acore.transpose(1, 0, 2).reshape(KC, 128, NFREE))

        idx = np.zeros(nchunk * 128, dtype=np.int64)
        mvals = np.zeros(nchunk * 128, dtype=np.int64)
        pairs = np.full((nchunk * 128, 2), -1, dtype=np.int64)
        goff = 0
        gcol_blocks = []
        for cb in range(NCB):
            for i, (c, r, gi, mp) in enumerate(core_groups[core][cb]):
                m = goff * 128 + i
                idx[m] = gi
                mvals[m] = mp
                pairs[m] = (c, r)
            nidx = gchunks[cb] * 128
            gi_arr = idx[goff * 128 : goff * 128 + nidx]
            blk = np.zeros((16, nidx // 16), dtype=np.int16)
            blk[np.arange(nidx) % 16, np.arange(nidx) // 16] = gi_arr
            gcol_blocks.append(np.tile(blk, (8, 1)))
            goff += gchunks[cb]
        gidx = np.ascontiguousarray(np.concatenate(gcol_blocks, axis=1))
        mdt = ml_dtypes.bfloat16 if MASK_BF16 else np.float32
        mask = np.zeros((128, nchunk, R), dtype=mdt)
        mv = mvals.reshape(nchunk, 128).T  # [128, nchunk]
        p_i, j_i = np.meshgrid(np.arange(128), np.arange(nchunk), indexing="ij")
        mask[p_i, j_i, mv] = 1.0

        m = {"gidx": gidx, "mask": mask}
        if MODE in ("float32", "float32r"):
            m["xt0"], m["ac0"] = xt_f32, ac_f32
        elif MODE == "bf16":
            m["xt0"] = xt_f32.astype(ml_dtypes.bfloat16)
            m["ac0"] = ac_f32.astype(ml_dtypes.bfloat16)
        else:  # split
            m["xt0"], m["xt1"] = _split_bf16(xt_f32)
            m["ac0"], m["ac1"] = _split_bf16(ac_f32)
        in_maps.append(m)
        pair_lists.append(pairs)

    nc = _build_bass_col(gchunks)
    return nc, in_maps, (pair_lists, nchunk)


def prepare(x, xids, wids, A):
    """Host-side planning + per-core input buffers. Returns (nc, in_maps, meta)."""
    if ORIENT == "col":
        return prepare_col(x, xids, wids, A)
    x = np.ascontiguousarray(np.asarray(x).reshape(BATCH, K), dtype=np.float32)
    xids = np.asarray(xids).astype(np.int64)
    wids = np.asarray(wids).astype(np.int64)
    A = np.ascontiguousarray(np.asarray(A), dtype=np.float32)

    core_slots = _plan(wids)
    tok = xids.reshape(C, R)

    # per-core (c, r, slot) lists grouped by token chunk, padded per group
    core_groups = []  # [core][t] -> list of (c, r, sl)
    for core in range(NCORES):
        groups = [[] for _ in range(TC)]
        for sl, (w, rows) in enumerate(core_slots[core]):
            for c in rows:
                for r in range(R):
                    groups[tok[c, r] // 128].append((c, r, sl))
        core_groups.append(groups)
    gchunks = [
        max(1, max(-(-len(core_groups[core][t]) // 128) for core in range(NCORES)))
        for t in range(TC)
    ]
    nchunk = sum(gchunks)

    xt_f32 = np.ascontiguousarray(x.T)  # [K, BATCH]

    in_maps, pair_lists = [], []
    for core in range(NCORES):
        ws = [w for w, _ in core_slots[core]]
        acore = A[ws]  # [NW, K, R] (duplicates for split adapters)
        ac_f32 = np.ascontiguousarray(acore.transpose(1, 0, 2).reshape(KC, 128, NFREE))

        idx = np.zeros(nchunk * 128, dtype=np.int64)
        rvals = np.zeros(nchunk * 128, dtype=np.int64)
        pairs = np.full((nchunk * 128, 2), -1, dtype=np.int64)
        goff = 0
        gcol_blocks = []
        for t in range(TC):
            for i, (c, r, sl) in enumerate(core_groups[core][t]):
                m = goff * 128 + i
                tt = tok[c, r]
                idx[m] = (tt % 128) * NW + sl
                rvals[m] = r
                pairs[m] = (c, r)
            # wrapped int16 layout for this group: [16, nidx/16] -> tile rows
            nidx = gchunks[t] * 128
            gi = idx[goff * 128 : goff * 128 + nidx]
            blk = np.zeros((16, nidx // 16), dtype=np.int16)
            blk[np.arange(nidx) % 16, np.arange(nidx) // 16] = gi
            gcol_blocks.append(np.tile(blk, (8, 1)))  # replicate to 128 rows
            goff += gchunks[t]
        gidx = np.ascontiguousarray(np.concatenate(gcol_blocks, axis=1))
        mdt = ml_dtypes.bfloat16 if MASK_BF16 else np.float32
        mask = np.zeros((128, nchunk, R), dtype=mdt)
        rv = rvals.reshape(nchunk, 128).T  # [128, nchunk]
        p_i, j_i = np.meshgrid(np.arange(128), np.arange(nchunk), indexing="ij")
        mask[p_i, j_i, rv] = 1.0

        m = {"gidx": gidx, "mask": mask}
        if MODE in ("float32", "float32r"):
            m["xt0"], m["ac0"] = xt_f32, ac_f32
        elif MODE == "bf16":
            m["xt0"] = xt_f32.astype(ml_dtypes.bfloat16)
            m["ac0"] = ac_f32.astype(ml_dtypes.bfloat16)
        else:  # split
            m["xt0"], m["xt1"] = _split_bf16(xt_f32)
            m["ac0"], m["ac1"] = _split_bf16(ac_f32)
        in_maps.append(m)
        pair_lists.append(pairs)

    nc = _build_bass(gchunks)
    return nc, in_maps, (pair_lists, nchunk)


def finish(results, meta):
    pair_lists, nchunk = meta
    out = np.zeros((C, 1, R), dtype=np.float32)
    for core in range(NCORES):
        vals = np.asarray(results[core]["out"]).T.reshape(-1)  # m = j*128+p
        pairs = pair_lists[core]
        sel = pairs[:, 0] >= 0
        out[pairs[sel, 0], 0, pairs[sel, 1]] = vals[sel]
    return out


def kernel(x, xids, wids, A):
    nc, in_maps, meta = prepare(x, xids, wids, A)
    res = run_bass_kernel_spmd(nc, in_maps, core_ids=list(range(NCORES)))
    return finish(res.results, meta)


if __name__ == "__main__":
    rng = np.random.default_rng(0)
    x = rng.standard_normal((BATCH, 1, K), dtype=np.float32)
    xids = rng.integers(0, BATCH, C * R).astype(np.int32)
    wids = rng.integers(0, NA, C).astype(np.int32)
    A = (rng.standard_normal((NA, K, R)) * 0.02).astype(np.float32)
    got = kernel(x=x, xids=xids, wids=wids, A=A)
    tokh = xids.reshape(C, R)
    want = np.einsum(
        "crk,ckr->cr",
        x[tokh, 0, :].astype(np.float64),
        A[wids].astype(np.float64),
    )[:, None, :]
    rel = np.abs(got - want).max() / np.abs(want).max()
    print(f"MODE={MODE} rel err vs f64: {rel:.3e}")

